# revision 1
# baseline (speedup 1.0000x reference)
"""Causal self-attention (B=2, T=2048, C=1024, H=16) on 8 trn2 NeuronCores.

Sharding: tensor-parallel over heads (2 heads/core) x both batches on every
core.  Each core computes, for its 2 heads:
  qkv^T slice  = (w_qkv slice)^T @ x^T        [384, 4096]
  flash-style causal attention in transposed (k-major) layout
  partial out-projection y_c = attn_out_c @ w_out[rows_c, :]   [4096, 1024]
Host sums the 8 partial y's and adds b_out.  No device collectives.

Matmul operands use float32r (PE full-rate fp32 mode, ~13 mantissa bits,
measured ~2.4e-4 end-to-end rel err) by default; mm_dt=float32 gives the
exact (4x slower) path.

PSUM plan (8 banks): tag "Sh" = 2 x [128,1024] (scores / transposes /
projection groups, double-buffered), tag "av" = 1 x [128,2048] (flash
accumulator with fused sumexp row; also qkv^T psum and spare yproj slot).

Self-contained: hardcodes all shapes; builds the Bass program once and
caches the compiled NEFF across calls.
"""

import numpy as np

import concourse.bass as bass
import concourse.mybir as mybir
import concourse.tile as tile
from concourse.bass_utils import run_bass_kernel_spmd

F32 = mybir.dt.float32
F32R = mybir.dt.float32r
AF = mybir.ActivationFunctionType

B, T, C = 2, 2048, 1024
NH, HD = 16, 64
BT = B * T              # 4096
NCORES = 8
HPC = NH // NCORES      # heads per core = 2
M3 = 3 * HPC * HD       # per-core qkv output cols = 384
TCH = 512               # t-chunk width for phase 1
NTC = BT // TCH         # 8
NKT = T // 128          # 16 k-tiles per (b, h)
NQC = T // 512          # 4 q-chunks per (b, h)
SCALE = 1.0 / 8.0       # 1/sqrt(HD)


def split_excess_waits(nc, max_waits=1):
    """Walrus in this env only accepts 1 sync-wait per instruction; move
    extras onto NoOps inserted right before the offending instruction."""
    for bb in nc.main_func.blocks:
        new_insts = []
        for ins in bb.instructions:
            si = ins.sync_info
            if si is not None and si.on_wait and len(si.on_wait) > max_waits:
                extra = list(si.on_wait[max_waits:])
                si.on_wait = list(si.on_wait[:max_waits])
                for i in range(0, len(extra), max_waits):
                    chunk = extra[i : i + max_waits]
                    nop = mybir.InstNoOp(
                        name=f"{ins.name}-wsplit-{i}",
                        ins=[],
                        outs=[],
                        sync_info=mybir.SyncInfo(on_wait=chunk, on_update=[]),
                    )
                    nop.engine = ins.engine
                    nc.register_instruction(nop)
                    new_insts.append(nop)
            new_insts.append(ins)
        bb.instructions[:] = new_insts


class Ctx:
    pass


def build(reps: int = 1, mm_dt=F32R):
    nc = bass.Bass()
    c = Ctx()
    c.mm_dt = mm_dt
    c.x = nc.declare_dram_parameter("x", [BT, C], F32, isOutput=False)
    c.w3 = nc.declare_dram_parameter("w3", [C, M3], F32, isOutput=False)
    c.b3 = nc.declare_dram_parameter("b3", [M3], F32, isOutput=False)
    c.wo = nc.declare_dram_parameter("wo", [HPC * HD, C], F32, isOutput=False)
    cmd = nc.declare_dram_parameter("cmask", [128, 128], F32, isOutput=False)
    idd = nc.declare_dram_parameter("ident", [128, 128], F32, isOutput=False)
    c.y = nc.declare_dram_parameter("y", [BT, C], F32, isOutput=True)

    with tile.TileContext(nc) as tc:
        with (
            tc.tile_pool(name="const", bufs=1) as cp,
            tc.tile_pool(name="xst", bufs=10) as xstp,
            tc.tile_pool(name="xt", bufs=8) as xtp,
            tc.tile_pool(name="exp", bufs=4) as expp,
            tc.tile_pool(name="rc", bufs=2) as rcp,
            tc.tile_pool(name="ysb", bufs=2) as ysbp,
            tc.tile_pool(name="ps", bufs=1, space="PSUM") as pp,
        ):
            c.pp, c.xstp, c.xtp, c.expp, c.rcp, c.ysbp = pp, xstp, xtp, expp, rcp, ysbp
            # ---- constants (staged f32, rounded to mm_dt where needed) ----
            c.ident = cp.tile([128, 128], F32, tag="ident")
            nc.sync.dma_start(c.ident[:], idd[:])
            c.identr = cp.tile([128, 128], mm_dt, tag="identr")
            nc.vector.tensor_copy(c.identr[:], c.ident[:])

            c.cmask = cp.tile([128, 128], mm_dt, tag="cmask")
            cmst = xstp.tile([128, 512], F32, tag="xst", name="cmst")
            nc.sync.dma_start(cmst[:, 0:128], cmd[:])
            nc.vector.tensor_copy(c.cmask[:], cmst[:, 0:128])

            c.w3sb = cp.tile([128, 8 * M3], mm_dt, tag="w3sb")
            for ci in range(8):
                w3st = xstp.tile([128, 512], F32, tag="xst", name=f"w3st{ci}")
                nc.sync.dma_start(w3st[:, 0:M3], c.w3[ci * 128 : (ci + 1) * 128, :])
                nc.vector.tensor_copy(
                    c.w3sb[:, ci * M3 : (ci + 1) * M3], w3st[:, 0:M3]
                )
            # w_out split per head, both halves at partition base 0
            c.woh = []
            for h in range(2):
                woh = cp.tile([64, C], mm_dt, tag=f"woh{h}", name=f"woh{h}")
                for hlf in range(2):
                    wst = xstp.tile([128, 512], F32, tag="xst",
                                    name=f"wost{h}_{hlf}")
                    nc.sync.dma_start(
                        wst[0:64, :],
                        c.wo[h * 64 : (h + 1) * 64, hlf * 512 : (hlf + 1) * 512],
                    )
                    nc.vector.tensor_copy(
                        woh[:, hlf * 512 : (hlf + 1) * 512], wst[0:64, :]
                    )
                c.woh.append(woh)

            c.b3sb = cp.tile([128, 3], F32, tag="b3sb")
            for mi in range(3):
                nc.sync.dma_start(
                    c.b3sb[:, mi : mi + 1],
                    c.b3[mi * 128 : (mi + 1) * 128].rearrange("(p o) -> p o", o=1),
                )
            # ones row at partition 64 (lane-aligned with the avps sumexp row)
            c.ones65 = cp.tile([65, 64], mm_dt, tag="ones65")
            c.onesc = cp.tile([128, 32], mm_dt, tag="onesc")
            onest = xstp.tile([128, 512], F32, tag="xst", name="onest")
            nc.vector.memset(onest[:, 0:64], 1.0)
            nc.vector.tensor_copy(c.ones65[64:65, :], onest[64:65, 0:64])
            nc.vector.tensor_copy(c.onesc[:], onest[:, 0:32])

            c.qT = cp.tile([128, BT], mm_dt, tag="qT")
            c.kT = cp.tile([128, BT], mm_dt, tag="kT")
            c.vT = cp.tile([128, BT], mm_dt, tag="vT")
            c.Vsb = [
                cp.tile([128, NKT * 2 * 65], mm_dt, tag=f"V{b}", name=f"Vsb{b}")
                for b in range(B)
            ]
            # per-(b, head) attn_out^T tiles, all at partition base 0
            c.attn = [
                [
                    cp.tile([64, T], mm_dt, tag=f"attn{b}{h}", name=f"attn{b}{h}")
                    for h in range(HPC)
                ]
                for b in range(B)
            ]

            for _rep in range(reps):
                emit_body(nc, c)

    split_excess_waits(nc)
    return nc


def emit_body(nc, c):
    mm_dt = c.mm_dt
    pp, xstp, xtp, expp, rcp, ysbp = c.pp, c.xstp, c.xtp, c.expp, c.rcp, c.ysbp

    # =========== Phase 1: x^T and qkv^T = w3^T @ x^T ===========
    for tc_i in range(NTC):
        t0 = tc_i * TCH
        xst = []
        for ti in range(4):
            row = []
            for hlf in range(2):
                xs = xstp.tile([128, 512], F32, tag="xst",
                               name=f"xs{tc_i}_{ti}_{hlf}")
                eng = nc.sync if (ti * 2 + hlf) % 2 == 0 else nc.scalar
                eng.dma_start(
                    xs[:],
                    c.x[t0 + ti * 128 : t0 + (ti + 1) * 128,
                        hlf * 512 : (hlf + 1) * 512],
                )
                row.append(xs)
            xst.append(row)
        # transpose (plain fp32) via [128,1024] Sh slots; round on evacuation
        xts = []
        for g in range(4):  # each group covers 2 c-chunks
            ps = pp.tile([128, 1024], F32, tag="Sh", name=f"xtp{tc_i}_{g}")
            for cl in range(2):
                ci = g * 2 + cl
                for ti in range(4):
                    nc.tensor.transpose(
                        ps[:, cl * 512 + ti * 128 : cl * 512 + (ti + 1) * 128],
                        xst[ti][ci // 4][:, (ci % 4) * 128 : (ci % 4 + 1) * 128],
                        c.ident[:],
                    )
            for cl in range(2):
                xt = xtp.tile([128, TCH], mm_dt, tag="xt",
                              name=f"xt{tc_i}_{g}_{cl}")
                nc.vector.tensor_copy(xt[:], ps[:, cl * 512 : (cl + 1) * 512])
                xts.append(xt)
        # qkv^T matmuls into the "av" mega-slot; bias-add evac on ACT
        q3 = pp.tile([128, 2048], F32, tag="av", name=f"q3_{tc_i}")
        for mi in range(3):
            for ci in range(8):
                nc.tensor.matmul(
                    q3[:, mi * 512 : (mi + 1) * 512],
                    c.w3sb[:, ci * M3 + mi * 128 : ci * M3 + (mi + 1) * 128],
                    xts[ci][:],
                    start=(ci == 0),
                    stop=(ci == 7),
                )
        for mi, dstT in enumerate([c.qT, c.kT, c.vT]):
            nc.scalar.activation(
                dstT[:, t0 : t0 + TCH],
                q3[:, mi * 512 : (mi + 1) * 512],
                AF.Identity,
                bias=c.b3sb[:, mi : mi + 1],
            )

    # ===== Phase 2: V natural layout [k, dv] per (b) with ones cols =====
    for b in range(B):
        vv = c.Vsb[b]
        nc.vector.tensor_copy(
            vv[:].rearrange("p (k d) -> p k d", d=65)[:, :, 64:65],
            c.onesc[:].unsqueeze(2),
        )
        for g in range(2):  # 8 k-tiles per group
            ps = pp.tile([128, 1024], mm_dt, tag="Sh", name=f"vtp{b}_{g}")
            for j in range(8):
                kt = g * 8 + j
                nc.tensor.transpose(
                    ps[:, j * 128 : (j + 1) * 128],
                    c.vT[:, b * T + kt * 128 : b * T + (kt + 1) * 128],
                    c.identr[:],
                )
            src = ps[:].rearrange("p (k h d) -> p k h d", k=8, h=2, d=64)
            dst = vv[:].rearrange("p (k h d) -> p k h d", k=NKT, h=2, d=65)[
                :, g * 8 : (g + 1) * 8, :, 0:64
            ]
            nc.vector.tensor_copy(dst, src)

    # ====== Phase 3: attention per (b, hh), q processed in 1024-halves ====
    pending_norms = []
    for b in range(B):
        for hh in range(HPC):
            p0 = hh * 64
            tb = b * T
            avps = pp.tile([65, 2048], F32, tag="av", name=f"av{b}_{hh}")
            if pending_norms:
                pending_norms.pop(0)()

            def emit_av(job):
                half, kt, qs0, et = job
                kb = kt * 128
                for qc in range(2 * half, 2 * half + 2):
                    qs = max(qc * 512, kb)
                    qe = (qc + 1) * 512
                    if qe <= kb:
                        continue
                    nc.tensor.matmul(
                        avps[:, qs:qe],
                        c.Vsb[b][:, kt * 130 + hh * 65 : kt * 130 + hh * 65 + 65],
                        et[:, qs - qs0 : qe - qs0],
                        start=(kt == 0 and half == (0 if qc < 2 else 1)),
                        stop=(kt == 4 * qc + 3),
                    )

            pend = None
            for half in range(2):
                qlo = half * 1024
                qhi = qlo + 1024
                for kt in range(min(8 * (half + 1), NKT)):
                    kb = kt * 128
                    qs0 = max(qlo, kb)
                    sps = pp.tile([128, 1024], F32, tag="Sh",
                                  name=f"sps{b}{hh}{half}_{kt}")
                    for qc in range(2 * half, 2 * half + 2):
                        qs = max(qc * 512, kb)
                        qe = (qc + 1) * 512
                        if qe <= kb:
                            continue
                        nc.tensor.matmul(
                            sps[:, qs - qlo : qe - qlo],
                            c.kT[p0 : p0 + 64, tb + kb : tb + kb + 128],
                            c.qT[p0 : p0 + 64, tb + qs : tb + qe],
                            start=True,
                            stop=True,
                        )
                    if pend is not None:
                        emit_av(pend)
                    et = expp.tile([128, 1024], mm_dt, tag="exp",
                                   name=f"et{b}{hh}{half}_{kt}")
                    nc.scalar.activation(
                        et[:, 0 : qhi - qs0], sps[:, qs0 - qlo : 1024],
                        AF.Exp, scale=SCALE,
                    )
                    if kb >= qlo:  # causal staircase at the diagonal
                        nc.gpsimd.tensor_mul(
                            et[:, 0:128], et[:, 0:128], c.cmask[:]
                        )
                    pend = (half, kt, qs0, et)
            emit_av(pend)
            # decouple: copy accumulator to SBUF (rounded), free "av" slot
            avsb = rcp.tile([65, 2048], mm_dt, tag="avsb", bufs=2,
                            name=f"avsb{b}{hh}")
            nc.vector.tensor_copy(avsb[:], avps[:])

            # lazy normalization, DEFERRED: emitted during the NEXT pair so
            # its Sh-slot (bc) allocations queue behind that pair's scores.
            def make_norm(b=b, hh=hh, avsb=avsb):
                def norm():
                    for g2 in range(2):
                        qcs = slice(g2 * 1024, (g2 + 1) * 1024)
                        bc = pp.tile([64, 1024], F32, tag="Sh",
                                     name=f"bc{b}{hh}{g2}")
                        for q5 in range(2):
                            nc.tensor.matmul(
                                bc[:, q5 * 512 : (q5 + 1) * 512],
                                c.ones65[64:65, :],
                                avsb[64:65,
                                     g2 * 1024 + q5 * 512 : g2 * 1024 + (q5 + 1) * 512],
                                start=True, stop=True,
                            )
                        rsb = rcp.tile([64, 1024], F32, tag="rc",
                                       name=f"rc{b}{hh}{g2}")
                        nc.vector.reciprocal(rsb[:], bc[:])
                        nc.vector.tensor_mul(
                            c.attn[b][hh][:, qcs], avsb[0:64, qcs], rsb[:]
                        )
                return norm
            pending_norms.append(make_norm())

    for nrm in pending_norms:
        nrm()
    pending_norms.clear()

    # =========== Phase 4: partial out-projection, both batches ===========
    u = 0
    for b in range(B):
        for tt in range(16):
            tag = "av" if u % 3 == 2 else "Sh"
            yps = pp.tile([128, 1024], F32, tag=tag, name=f"yps{b}_{tt}")
            for no in range(2):
                for h in range(2):
                    nc.tensor.matmul(
                        yps[:, no * 512 : (no + 1) * 512],
                        c.attn[b][h][:, tt * 128 : (tt + 1) * 128],
                        c.woh[h][:, no * 512 : (no + 1) * 512],
                        start=(h == 0),
                        stop=(h == 1),
                    )
            ysb = ysbp.tile([128, 1024], F32, tag="ysb", name=f"ysb{b}_{tt}")
            if u % 2 == 0:
                nc.vector.tensor_copy(ysb[:], yps[:])
            else:
                nc.scalar.copy(ysb[:], yps[:])
            seng = nc.sync if u % 2 == 0 else nc.scalar
            seng.dma_start(
                c.y[b * T + tt * 128 : b * T + (tt + 1) * 128, :], ysb[:]
            )
            u += 1


_CACHE: dict = {}


def _get_nc(reps: int = 1, mm_dt=F32R):
    key = ("nc", reps, str(mm_dt))
    if key not in _CACHE:
        _CACHE[key] = build(reps, mm_dt)
    return _CACHE[key]


def make_in_maps(x, w_qkv, b_qkv, w_out):
    x2d = np.ascontiguousarray(np.asarray(x, np.float32).reshape(BT, C))
    w_qkv = np.asarray(w_qkv, np.float32)
    b_qkv = np.asarray(b_qkv, np.float32)
    w_out = np.asarray(w_out, np.float32)
    cmask = np.triu(np.ones((128, 128), np.float32))
    ident = np.eye(128, dtype=np.float32)
    in_maps = []
    for c in range(NCORES):
        h0 = c * HPC                      # first head on this core
        col0 = h0 * HD                    # = c*128
        w3 = np.ascontiguousarray(
            np.concatenate(
                [w_qkv[:, s * C + col0 : s * C + col0 + HPC * HD] for s in range(3)],
                axis=1,
            )
        )
        b3 = np.ascontiguousarray(
            np.concatenate(
                [b_qkv[s * C + col0 : s * C + col0 + HPC * HD] for s in range(3)]
            )
        )
        woc = np.ascontiguousarray(w_out[col0 : col0 + HPC * HD, :])
        in_maps.append(
            {"x": x2d, "w3": w3, "b3": b3, "wo": woc, "cmask": cmask, "ident": ident}
        )
    return in_maps


def kernel(x, w_qkv, b_qkv, w_out, b_out):
    nc = _get_nc()
    in_maps = make_in_maps(x, w_qkv, b_qkv, w_out)
    res = run_bass_kernel_spmd(nc, in_maps, core_ids=list(range(NCORES)))
    acc = np.zeros((BT, C), np.float64)
    for c in range(NCORES):
        acc += res.results[c]["y"].astype(np.float64)
    out = acc.astype(np.float32) + np.asarray(b_out, np.float32)[None, :]
    return out.reshape(B, T, C)

